# revision 17
# baseline (speedup 1.0000x reference)
"""GAT (2-layer) Trainium2 Bass kernel, 8-core SPMD — fused single-launch.

v3 design (axon tunnel is ~80MB/s up / ~50MB/s down, so bytes-over-tunnel and
per-call recompilation dominate; device exec is ~ms):
- ONE fused device program runs both GAT layers: L1 projection (node-sharded)
  -> AllGather h1 table -> L1 edge pass (dst-sharded scatter via one-hot
  matmuls) -> ELU -> L2 projection (+ attention src/dst terms as extra table
  columns) -> AllGather h2 table -> L2 edge pass -> fp16 output shard.
- ALL attention logits are computed on device. L1: asn1/adn1 are extra
  projection columns stored in a second gather table (asntab1, 256B rows);
  asn1[src] comes via the src gather, adn1[dst] via a local-table gather
  keyed by dst. L2: asn2/adn2 ride in the h2 table rows (cols 64/65). The
  raw per-edge attr ew (shared by both layers) is the only per-edge float
  upload (fp16); per-head We*ae coefficients arrive via a tiny cscal tensor.
- Empty padding slots carry dstloc=255: their one-hot scatter column matches
  nothing, so phantom edges contribute exactly zero regardless of logits.
- Edge scheduling: edges sharded by dst node-range; per core ordered by
  (src 32k-window group, dst 128-block); padded to a compile-time-identical
  tile/chunk structure across cores. Structure + compiled program + jitted
  executable + host staging buffers are cached across calls keyed on a hash
  of (edge_index, edge_weight).
- Per-call tunnel traffic: ~45MB up, ~13MB down (vs ~530MB/76MB for the
  two-launch replicated design). Warm wall-clock ~1.0s vs 25.3s baseline.
"""

import time
import hashlib
import numpy as np
from contextlib import ExitStack

import jax
import jax.numpy as jnp
from jax.sharding import Mesh, PartitionSpec, NamedSharding

from jax.experimental.shard_map import shard_map

import concourse.bass as bass
import concourse.tile as tile
from concourse import bacc, mybir, bass2jax

F32 = mybir.dt.float32
BF16 = mybir.dt.bfloat16
F16 = mybir.dt.float16
I16 = mybir.dt.int16
AF = mybir.ActivationFunctionType
ALU = mybir.AluOpType

BF16_NP = np.dtype("bfloat16")

NCORES = 8
P = 128
CHUNK = 8192
SLOTS = CHUNK // P          # 64
IDXF = CHUNK // 16          # 512
SRC_CHUNK = 32768
NEG = -60000.0              # "minus infinity" that fits fp16

# problem constants
HID = 64
OUT_DIM = 64
H1, H2 = 2, 1

LAST_EXEC_NS = None


def _ceil_to(x, m):
    return (x + m - 1) // m * m


# --------------------------------------------------------------------------
# host: graph structure (cached per edge_index)
# --------------------------------------------------------------------------

def build_structure(src, dst, n):
    """Vectorized edge scheduling. Returns dict with compile-time structure
    and ready-to-upload concat arrays for everything except per-layer edge
    values (al1/ewt), for which a scatter index is provided."""
    NS = n // NCORES
    assert NS * NCORES == n
    NSP = _ceil_to(NS, P)
    NB = NSP // P
    TBL = NCORES * NSP
    NG = (TBL + SRC_CHUNK - 1) // SRC_CHUNK
    E = src.shape[0]

    tid = (src // NS) * NSP + (src % NS)          # gather-table row of src
    grp = tid // SRC_CHUNK
    core = dst // NS
    dloc = dst % NS
    blk = dloc // P

    key = (core * NG + grp) * NB + blk
    counts = np.bincount(key, minlength=NCORES * NG * NB)
    cnt3 = counts.reshape(NCORES, NG, NB)
    tiles_gb = (-(-cnt3 // P)).max(axis=0)        # [NG, NB]
    ts = np.cumsum(tiles_gb, axis=1) - tiles_gb   # tile start within group
    gtiles = tiles_gb.sum(axis=1)
    gchunks = -(-gtiles // SLOTS)
    first_chunk = np.concatenate([[0], np.cumsum(gchunks)[:-1]])
    NC = int(gchunks.sum())

    order = np.argsort(key, kind="stable")
    ro = np.empty(E, np.int64)
    ro[order] = np.arange(E)
    bucket_start = np.concatenate([[0], np.cumsum(counts)])
    j = ro - bucket_start[key]                    # rank within (c,g,b) bucket
    slot = first_chunk[grp] * CHUNK + (ts[grp, blk] + j // P) * P + (j % P)

    FL = NC * CHUNK
    gsrc_flat = np.zeros((NCORES, FL), np.int16)
    gsrc_flat[core, slot] = (tid - grp * SRC_CHUNK).astype(np.int16)
    gdst_flat = np.zeros((NCORES, FL), np.int16)
    gdst_flat[core, slot] = dloc.astype(np.int16)
    # 255 = empty-slot sentinel: the one-hot scatter column matches nothing,
    # so phantom (padding) edges contribute zero regardless of their logits
    dstloc_flat = np.full((NCORES, FL), 255, np.uint8)
    dstloc_flat[core, slot] = (dloc % P).astype(np.uint8)

    def wrap_scalar(flat, width, dtype):
        a = flat.reshape(NCORES, NC, SLOTS, P, width)
        return np.ascontiguousarray(
            a.transpose(0, 3, 1, 2, 4).reshape(NCORES * P, NC * SLOTS * width)
        ).astype(dtype)

    def wrap_idx(flat):
        a = flat.reshape(NCORES, NC, IDXF, 16)
        return np.ascontiguousarray(
            a.transpose(0, 3, 1, 2).reshape(NCORES * 16, NC * IDXF))

    gsrc_c = wrap_idx(gsrc_flat)
    gdst_c = wrap_idx(gdst_flat)
    dstloc_c = wrap_scalar(dstloc_flat, 1, np.uint8)

    # per-edge row into the wrapped [NCORES*P*NC*SLOTS, width] layout
    k = slot // CHUNK
    r = slot % CHUNK
    s = r // P
    p = r % P
    wal_row = (core * P + p) * (NC * SLOTS) + k * SLOTS + s

    # compile-time tile descriptors
    chunk_group, chunk_tiles = [], []
    for q in range(NG):
        gts = []
        for b in range(NB):
            t = int(tiles_gb[q, b])
            for i in range(t):
                gts.append((b, i == 0, i == t - 1))
        gts += [None] * ((-len(gts)) % SLOTS)
        for i0 in range(0, len(gts), SLOTS):
            chunk_group.append(q)
            chunk_tiles.append(gts[i0:i0 + SLOTS])
    assert len(chunk_tiles) == NC

    sig = hashlib.blake2b(
        (repr((NS, NSP, NB, TBL, NG, NC, chunk_group, chunk_tiles))).encode(),
        digest_size=16).hexdigest()

    return dict(NS=NS, NSP=NSP, NB=NB, TBL=TBL, NG=NG, NC=NC,
                chunk_group=chunk_group, chunk_tiles=chunk_tiles,
                gsrc_c=gsrc_c, gdst_c=gdst_c, dstloc_c=dstloc_c,
                wal_row=wal_row, sig=sig)


_STRUCT_CACHE = {}


def get_structure(edge_index, ew, n):
    h = hashlib.blake2b(np.ascontiguousarray(edge_index).tobytes(),
                        digest_size=16)
    h.update(np.ascontiguousarray(ew).tobytes())
    hkey = h.digest() + n.to_bytes(8, "little")
    ent = _STRUCT_CACHE.get(hkey)
    if ent is None:
        src, dst, ea = add_self_loops_np(
            np.asarray(edge_index[0], np.int64),
            np.asarray(edge_index[1], np.int64), ew, n)
        S = build_structure(src, dst, n)
        S["src"], S["dst"], S["ea"] = src, dst, ea
        _STRUCT_CACHE[hkey] = S
        ent = S
    return ent


def add_self_loops_np(src, dst, ew, n):
    deg = np.bincount(dst, minlength=n).astype(np.float32)
    sw = np.bincount(dst, weights=ew[:, 0], minlength=n).astype(np.float32)
    loop = sw / np.maximum(deg, 1.0)
    ar = np.arange(n, dtype=src.dtype)
    return (np.concatenate([src, ar]), np.concatenate([dst, ar]),
            np.concatenate([ew, loop[:, None].astype(np.float32)], axis=0))


# --------------------------------------------------------------------------
# device program (fused 2 layers)
# --------------------------------------------------------------------------

def build_program(S):
    NSP, NB, TBL, NC = S["NSP"], S["NB"], S["TBL"], S["NC"]
    chunk_group, chunk_tiles = S["chunk_group"], S["chunk_tiles"]
    HC1 = H1 * HID            # 128
    HC2 = H2 * OUT_DIM        # 64
    RW1 = HC1 + H1            # 130
    RW2 = HC2 + H2            # 65

    nc = bacc.Bacc("TRN2", target_bir_lowering=False, debug=False,
                   num_devices=NCORES)

    xT = nc.dram_tensor("xT", [P, NSP], BF16, kind="ExternalInput")
    # W1e = [W1 | Ws1 (H1 cols) | Wd1 (H1 cols)] so the L1 projection also
    # produces per-node attention terms asn1/adn1
    W1e = nc.dram_tensor("W1e", [P, HC1 + 2 * H1], BF16, kind="ExternalInput")
    W2e = nc.dram_tensor("W2e", [P, HC2 + 2], BF16, kind="ExternalInput")
    b1r = nc.dram_tensor("b1r", [P, HC1], F32, kind="ExternalInput")
    b2r = nc.dram_tensor("b2r", [P, HC2], F32, kind="ExternalInput")
    iotaT = nc.dram_tensor("iotaT", [P, P], BF16, kind="ExternalInput")
    ident = nc.dram_tensor("ident", [P, P], BF16, kind="ExternalInput")
    cscal = nc.dram_tensor("cscal", [P, 4], F32, kind="ExternalInput")
    ew = nc.dram_tensor("ew", [P, NC * SLOTS], F16, kind="ExternalInput")
    dstl = nc.dram_tensor("dstl", [P, NC * SLOTS], mybir.dt.uint8,
                      kind="ExternalInput")
    gsrc = nc.dram_tensor("gsrc", [16, NC * IDXF], I16, kind="ExternalInput")
    gdst = nc.dram_tensor("gdst", [16, NC * IDXF], I16, kind="ExternalInput")
    out = nc.dram_tensor("out", [NSP, HC2], F16, kind="ExternalOutput")

    htab1s = nc.dram_tensor("htab1s", [NSP, 128], BF16, kind="Internal")
    htab1 = nc.dram_tensor("htab1", [TBL, 128], BF16, kind="Internal",
                           addr_space="Shared")
    asntab1s = nc.dram_tensor("asntab1s", [NSP, 128], BF16, kind="Internal")
    asntab1 = nc.dram_tensor("asntab1", [TBL, 128], BF16, kind="Internal",
                             addr_space="Shared")
    htab2s = nc.dram_tensor("htab2s", [NSP, 128], BF16, kind="Internal")
    htab2 = nc.dram_tensor("htab2", [TBL, 128], BF16, kind="Internal",
                           addr_space="Shared")
    eluD = nc.dram_tensor("eluD", [NSP, HC1], BF16, kind="Internal")
    gsrcr = nc.dram_tensor("gsrcr", [P, NC * IDXF], I16, kind="Internal")
    gdstr = nc.dram_tensor("gdstr", [P, NC * IDXF], I16, kind="Internal")

    with ExitStack() as ctx:
        tc = ctx.enter_context(tile.TileContext(nc))
        cpool = ctx.enter_context(tc.tile_pool(name="const", bufs=1))
        W1_sb = cpool.tile([P, HC1 + 2 * H1], BF16)
        nc.sync.dma_start(W1_sb[:], W1e.ap())
        W2e_sb = cpool.tile([P, HC2 + 2], BF16)
        nc.sync.dma_start(W2e_sb[:], W2e.ap())
        b1_sb = cpool.tile([P, 1, HC1], F32)
        nc.sync.dma_start(b1_sb[:, 0, :], b1r.ap())
        b2_sb = cpool.tile([P, 1, HC2], F32)
        nc.sync.dma_start(b2_sb[:, 0, :], b2r.ap())
        iota_sb = cpool.tile([P, 1, P], BF16)
        nc.sync.dma_start(iota_sb[:, 0, :], iotaT.ap())
        id_sb = cpool.tile([P, P], BF16)
        nc.sync.dma_start(id_sb[:], ident.ap())
        csc_sb = cpool.tile([P, 1, 4], F32)
        nc.sync.dma_start(csc_sb[:, 0, :], cscal.ap())
        acc_sb = cpool.tile([P, NB, RW1], F32)

        # replicate compact idx arrays to 128 partitions (DRAM->DRAM)
        for rr in range(8):
            nc.sync.dma_start(gsrcr.ap()[rr * 16:(rr + 1) * 16, :], gsrc.ap())
            nc.sync.dma_start(gdstr.ap()[rr * 16:(rr + 1) * 16, :], gdst.ap())

        xpool = ctx.enter_context(tc.tile_pool(name="xp", bufs=3))
        hpool = ctx.enter_context(tc.tile_pool(name="hp", bufs=3))
        pspool = ctx.enter_context(tc.tile_pool(name="ps", bufs=4,
                                                space="PSUM"))
        ipool = ctx.enter_context(tc.tile_pool(name="ip", bufs=3))
        gpool = ctx.enter_context(tc.tile_pool(name="gp", bufs=3))
        apool = ctx.enter_context(tc.tile_pool(name="apl", bufs=3))
        epool = ctx.enter_context(tc.tile_pool(name="ep", bufs=3))
        spool = ctx.enter_context(tc.tile_pool(name="sp", bufs=2))
        rpool = ctx.enter_context(tc.tile_pool(name="rp", bufs=2))
        mpool = ctx.enter_context(tc.tile_pool(name="mp", bufs=4,
                                               space="PSUM"))
        fpool = ctx.enter_context(tc.tile_pool(name="fp", bufs=2))

        # ---------------- phase A: L1 projection of own shard --------------
        # h1ext = x @ [W1|Ws1|Wd1]: cols 0:HC1 -> htab1s, HC1:HC1+4
        # (asn1/adn1 per head) -> asntab1s
        BK = 4
        NA = 2 * H1
        for b0 in range(0, NB, BK):
            kk = min(BK, NB - b0)
            xt = xpool.tile([P, BK * P], BF16, tag="xt", bufs=2)
            nc.sync.dma_start(xt[:, 0:kk * P], xT.ap()[:, b0 * P:(b0 + kk) * P])
            hs = hpool.tile([P, BK, 128], BF16, tag="hs", bufs=2)
            asnt = hpool.tile([P, BK, 128], BF16, tag="asnt", bufs=2)
            nc.vector.memset(asnt[:, 0:kk, NA:128], 0.0)
            for i in range(kk):
                ps = pspool.tile([P, HC1 + NA], F32, tag="ps", bufs=4)
                nc.tensor.matmul(ps[:], xt[:, i * P:(i + 1) * P], W1_sb[:],
                                 start=True, stop=True)
                nc.scalar.activation(hs[:, i, :], ps[:, 0:HC1], AF.Copy)
                nc.scalar.activation(asnt[:, i, 0:NA], ps[:, HC1:HC1 + NA],
                                     AF.Copy)
            nc.sync.dma_start(
                htab1s.ap()[b0 * P:(b0 + kk) * P, :].rearrange(
                    "(k p) t -> p k t", p=P),
                hs[:, 0:kk, :])
            nc.sync.dma_start(
                asntab1s.ap()[b0 * P:(b0 + kk) * P, :].rearrange(
                    "(k p) t -> p k t", p=P),
                asnt[:, 0:kk, :])

        nc.gpsimd.collective_compute(
            "AllGather", ALU.bypass,
            replica_groups=[list(range(NCORES))],
            ins=[htab1s[:].opt()], outs=[htab1[:].opt()])
        nc.gpsimd.collective_compute(
            "AllGather", ALU.bypass,
            replica_groups=[list(range(NCORES))],
            ins=[asntab1s[:].opt()], outs=[asntab1[:].opt()])

        # ---------------- edge pass helper ---------------------------------
        def edge_phase(layer):
            hc = HC1 if layer == 1 else HC2
            nh = H1 if layer == 1 else H2
            hd = hc // nh
            rw = hc + nh
            tab = htab1 if layer == 1 else htab2
            cur = [None]

            def close_run():
                if cur[0] is not None:
                    pst, bb = cur[0]
                    nc.vector.tensor_add(acc_sb[:, bb, 0:rw],
                                         acc_sb[:, bb, 0:rw], pst[:, 0:rw])
                    cur[0] = None

            for ck in range(NC):
                q = chunk_group[ck]
                r0 = q * SRC_CHUNK
                r1 = min(r0 + SRC_CHUNK, TBL)
                gi = ipool.tile([P, IDXF], I16, tag="gi", bufs=3)
                nc.sync.dma_start(gi[:],
                                  gsrcr.ap()[:, ck * IDXF:(ck + 1) * IDXF])
                grows = gpool.tile([P, SLOTS, 128], BF16, tag="grows", bufs=3)
                nc.gpsimd.dma_gather(grows[:], tab.ap()[r0:r1, :], gi[:],
                                     num_idxs=CHUNK, num_idxs_reg=CHUNK,
                                     elem_size=128, single_packet=False)
                e16 = apool.tile([P, SLOTS, 1], F16)
                nc.sync.dma_start(
                    e16[:, :, 0], ew.ap()[:, ck * SLOTS:(ck + 1) * SLOTS])
                ewf = epool.tile([P, SLOTS, 1], F32)
                nc.vector.tensor_copy(ewf[:], e16[:])
                if layer == 1:
                    # asn1[src] via src-gather of asntab1; adn1[dst] via
                    # local-table gather keyed by dst
                    ga_s = gpool.tile([P, SLOTS, 128], BF16, tag="grows",
                                      bufs=3)
                    nc.gpsimd.dma_gather(ga_s[:], asntab1.ap()[r0:r1, :],
                                         gi[:], num_idxs=CHUNK,
                                         num_idxs_reg=CHUNK, elem_size=128,
                                         single_packet=False)
                    gid = ipool.tile([P, IDXF], I16, tag="gi", bufs=3)
                    nc.sync.dma_start(
                        gid[:], gdstr.ap()[:, ck * IDXF:(ck + 1) * IDXF])
                    ga_d = gpool.tile([P, SLOTS, 128], BF16, tag="grows",
                                      bufs=3)
                    nc.gpsimd.dma_gather(ga_d[:], asntab1s.ap()[0:NSP, :],
                                         gid[:], num_idxs=CHUNK,
                                         num_idxs_reg=CHUNK, elem_size=128,
                                         single_packet=False)
                    alf = epool.tile([P, SLOTS, nh], F32)
                    nc.vector.tensor_copy(alf[:], ga_s[:, :, 0:nh])
                    t_adn = epool.tile([P, SLOTS, nh], F32)
                    nc.vector.tensor_copy(t_adn[:], ga_d[:, :, nh:2 * nh])
                    nc.vector.tensor_add(alf[:], alf[:], t_adn[:])
                    ewc = epool.tile([P, SLOTS, nh], F32)
                    e1a, e2a = bass.broadcast_tensor_aps(
                        ewf[:], csc_sb[:, :, 0:nh])
                    nc.vector.tensor_mul(ewc[:], e1a, e2a)
                    nc.vector.tensor_add(alf[:], alf[:], ewc[:])
                else:
                    gid = ipool.tile([P, IDXF], I16, tag="gi", bufs=3)
                    nc.sync.dma_start(
                        gid[:], gdstr.ap()[:, ck * IDXF:(ck + 1) * IDXF])
                    growsd = gpool.tile([P, SLOTS, 128], BF16, tag="grows", bufs=3)
                    nc.gpsimd.dma_gather(growsd[:], htab2s.ap()[0:NSP, :],
                                         gid[:], num_idxs=CHUNK,
                                         num_idxs_reg=CHUNK, elem_size=128,
                                         single_packet=False)
                    alf = epool.tile([P, SLOTS, 1], F32)
                    e1a, e2a = bass.broadcast_tensor_aps(
                        ewf[:], csc_sb[:, :, 2:3])
                    nc.vector.tensor_mul(alf[:], e1a, e2a)
                    t_asn = epool.tile([P, SLOTS, 1], F32)
                    nc.vector.tensor_copy(t_asn[:], grows[:, :, hc:hc + 1])
                    nc.vector.tensor_add(alf[:], alf[:], t_asn[:])
                    t_adn = epool.tile([P, SLOTS, 1], F32)
                    nc.vector.tensor_copy(t_adn[:], growsd[:, :, hc + 1:hc + 2])
                    nc.vector.tensor_add(alf[:], alf[:], t_adn[:])

                # ex = exp(leaky_relu(al))
                t1 = epool.tile([P, SLOTS, nh], F32)
                nc.vector.tensor_scalar_mul(t1[:], alf[:], 0.2)
                nc.vector.tensor_max(t1[:], t1[:], alf[:])
                ex = epool.tile([P, SLOTS, nh], BF16)
                nc.scalar.activation(ex[:], t1[:], AF.Exp)

                dl8 = apool.tile([P, SLOTS, 1], mybir.dt.uint8)
                nc.sync.dma_start(dl8[:, :, 0],
                                  dstl.ap()[:, ck * SLOTS:(ck + 1) * SLOTS])
                dlt = apool.tile([P, SLOTS, 1], BF16)
                nc.vector.tensor_copy(dlt[:], dl8[:])
                sw = spool.tile([P, SLOTS, P], BF16)
                a1, a2 = bass.broadcast_tensor_aps(iota_sb[:], dlt[:])
                nc.vector.tensor_tensor(sw[:], a1, a2, ALU.is_equal)

                rhs = rpool.tile([P, SLOTS, rw], BF16)
                for h in range(nh):
                    b1a, b2a = bass.broadcast_tensor_aps(
                        grows[:, :, h * hd:(h + 1) * hd], ex[:, :, h:h + 1])
                    nc.vector.tensor_mul(rhs[:, :, h * hd:(h + 1) * hd],
                                         b1a, b2a)
                nc.vector.tensor_copy(rhs[:, :, hc:hc + nh], ex[:])

                for s in range(SLOTS):
                    td = chunk_tiles[ck][s]
                    if td is None:
                        continue
                    bb, st, sp = td
                    if st:
                        close_run()
                        pst = mpool.tile([P, RW1], F32, tag="pst", bufs=4)
                        cur[0] = (pst, bb)
                    else:
                        pst, _ = cur[0]
                    nc.tensor.matmul(pst[:, 0:rw], sw[:, s, :], rhs[:, s, :],
                                     start=st, stop=sp)
            close_run()

        # ---------------- phase B: L1 edges --------------------------------
        nc.vector.memset(acc_sb[:], 0.0)
        edge_phase(1)

        # finalize L1: out1 = num/den + b1, ELU, -> eluD (bf16)
        FB = 4
        for b0 in range(0, NB, FB):
            kf = min(FB, NB - b0)
            rec = fpool.tile([P, FB, H1], F32)
            nc.vector.tensor_scalar_add(
                rec[:, 0:kf, :], acc_sb[:, b0:b0 + kf, HC1:HC1 + H1], 1e-30)
            nc.vector.reciprocal(rec[:, 0:kf, :], rec[:, 0:kf, :])
            outt = fpool.tile([P, FB, HC1], F32)
            for h in range(H1):
                c1, c2 = bass.broadcast_tensor_aps(
                    acc_sb[:, b0:b0 + kf, h * HID:(h + 1) * HID],
                    rec[:, 0:kf, h:h + 1])
                nc.vector.tensor_mul(outt[:, 0:kf, h * HID:(h + 1) * HID],
                                     c1, c2)
            d1, d2 = bass.broadcast_tensor_aps(outt[:, 0:kf, :], b1_sb[:])
            nc.vector.tensor_add(outt[:, 0:kf, :], d1, d2)
            # ELU
            neg = fpool.tile([P, FB, HC1], F32)
            nc.vector.tensor_scalar_min(neg[:, 0:kf, :], outt[:, 0:kf, :], 0.0)
            enx = fpool.tile([P, FB, HC1], F32)
            nc.scalar.activation(enx[:, 0:kf, :], neg[:, 0:kf, :], AF.Exp)
            nc.vector.tensor_scalar_add(enx[:, 0:kf, :], enx[:, 0:kf, :], -1.0)
            nc.vector.tensor_scalar_max(outt[:, 0:kf, :], outt[:, 0:kf, :],
                                        0.0)
            nc.vector.tensor_add(outt[:, 0:kf, :], outt[:, 0:kf, :],
                                 enx[:, 0:kf, :])
            e16 = fpool.tile([P, FB, HC1], BF16)
            nc.vector.tensor_copy(e16[:, 0:kf, :], outt[:, 0:kf, :])
            nc.sync.dma_start(
                eluD.ap()[b0 * P:(b0 + kf) * P, :].rearrange(
                    "(k p) c -> p k c", p=P),
                e16[:, 0:kf, :])

        # ---------------- phase C: L2 projection ---------------------------
        for b in range(NB):
            et = xpool.tile([P, P], BF16)
            nc.sync.dma_start(
                et[:],
                eluD.ap()[b * P:(b + 1) * P, :].rearrange(
                    "(k p) c -> p (k c)", p=P))
            psT = pspool.tile([P, 128], F32, tag="ps", bufs=4)
            nc.tensor.matmul(psT[:], et[:], id_sb[:], start=True, stop=True)
            etT = hpool.tile([P, P], BF16)
            nc.scalar.activation(etT[:], psT[:], AF.Copy)
            ps2 = pspool.tile([P, 128], F32, tag="ps", bufs=4)
            nc.tensor.matmul(ps2[:, 0:HC2 + 2], etT[:], W2e_sb[:],
                             start=True, stop=True)
            hs2 = hpool.tile([P, 128], BF16)
            nc.vector.memset(hs2[:, HC2 + 2:128], 0.0)
            nc.scalar.activation(hs2[:, 0:HC2 + 2], ps2[:, 0:HC2 + 2],
                                 AF.Copy)
            nc.sync.dma_start(
                htab2s.ap()[b * P:(b + 1) * P, :].rearrange(
                    "(k p) t -> p (k t)", p=P),
                hs2[:])

        nc.gpsimd.collective_compute(
            "AllGather", ALU.bypass,
            replica_groups=[list(range(NCORES))],
            ins=[htab2s[:].opt()], outs=[htab2[:].opt()])

        # ---------------- phase D: L2 edges --------------------------------
        nc.vector.memset(acc_sb[:, :, 0:RW2], 0.0)
        edge_phase(2)

        # finalize L2: out = num/den + b2 -> fp16
        for b0 in range(0, NB, FB):
            kf = min(FB, NB - b0)
            rec = fpool.tile([P, FB, H2], F32)
            nc.vector.tensor_scalar_add(
                rec[:, 0:kf, :], acc_sb[:, b0:b0 + kf, HC2:HC2 + H2], 1e-30)
            nc.vector.reciprocal(rec[:, 0:kf, :], rec[:, 0:kf, :])
            outt = fpool.tile([P, FB, HC2], F32)
            c1, c2 = bass.broadcast_tensor_aps(
                acc_sb[:, b0:b0 + kf, 0:HC2], rec[:, 0:kf, 0:1])
            nc.vector.tensor_mul(outt[:, 0:kf, :], c1, c2)
            d1, d2 = bass.broadcast_tensor_aps(outt[:, 0:kf, :], b2_sb[:])
            nc.vector.tensor_add(outt[:, 0:kf, :], d1, d2)
            o16 = fpool.tile([P, FB, HC2], F16)
            nc.vector.tensor_copy(o16[:, 0:kf, :], outt[:, 0:kf, :])
            nc.sync.dma_start(
                out.ap()[b0 * P:(b0 + kf) * P, :].rearrange(
                    "(k p) c -> p k c", p=P),
                o16[:, 0:kf, :])

    nc.compile()
    return nc


# --------------------------------------------------------------------------
# persistent-jit runner
# --------------------------------------------------------------------------

class Runner:
    def __init__(self, nc):
        bass2jax.install_neuronx_cc_hook()
        self.nc = nc
        partition_name = (nc.partition_id_tensor.name
                          if nc.partition_id_tensor else None)
        in_names, out_names, out_avals = [], [], []
        for alloc in nc.m.functions[0].allocations:
            if not isinstance(alloc, mybir.MemoryLocationSet):
                continue
            name = alloc.memorylocations[0].name
            if alloc.kind == "ExternalInput":
                if name != partition_name:
                    in_names.append(name)
            elif alloc.kind == "ExternalOutput":
                out_names.append(name)
                out_avals.append(jax.core.ShapedArray(
                    tuple(alloc.tensor_shape), mybir.dt.np(alloc.dtype)))
        self.in_names = in_names
        self.out_names = out_names
        self.out_avals = out_avals
        n_params = len(in_names)
        n_outs = len(out_avals)
        all_in = list(in_names) + list(out_names)
        if partition_name is not None:
            all_in.append(partition_name)
        donate = tuple(range(n_params, n_params + n_outs))

        def _body(*args):
            operands = list(args)
            if partition_name is not None:
                operands.append(bass2jax.partition_id_tensor())
            outs = bass2jax._bass_exec_p.bind(
                *operands,
                out_avals=tuple(out_avals),
                in_names=tuple(all_in),
                out_names=tuple(out_names),
                lowering_input_output_aliases=(),
                sim_require_finite=True,
                sim_require_nnan=True,
                nc=nc,
            )
            return tuple(outs)

        devices = jax.devices()[:NCORES]
        self.mesh = Mesh(np.asarray(devices), ("core",))
        self.sharding = NamedSharding(self.mesh, PartitionSpec("core"))
        in_specs = (PartitionSpec("core"),) * (n_params + n_outs)
        out_specs = (PartitionSpec("core"),) * n_outs
        self.fn = jax.jit(
            shard_map(_body, mesh=self.mesh, in_specs=in_specs,
                      out_specs=out_specs, check_rep=False),
            donate_argnums=donate, keep_unused=True)
        self._zeros_fns = [
            jax.jit(lambda av=av: jnp.zeros((NCORES * av.shape[0],)
                                            + av.shape[1:], av.dtype),
                    out_shardings=self.sharding)
            for av in out_avals]

    def put(self, arr):
        """Async host->device upload with core sharding on axis 0."""
        return jax.device_put(arr, self.sharding)

    def run(self, concat_map):
        args = [concat_map[nm] for nm in self.in_names]
        zeros = [zf() for zf in self._zeros_fns]
        outs = self.fn(*args, *zeros)
        return {nm: outs[i] for i, nm in enumerate(self.out_names)}


_PROG_CACHE = {}
_HOST_BUFS = {}


def _hbuf(key, shape, dtype):
    """Reusable host staging buffer (avoids per-call alloc + page faults)."""
    b = _HOST_BUFS.get(key)
    if b is None or b.shape != tuple(shape) or b.dtype != dtype:
        b = np.empty(shape, dtype)
        _HOST_BUFS[key] = b
    return b


def get_runner(S):
    key = S["sig"]
    ent = _PROG_CACHE.get(key)
    if ent is None:
        nc = build_program(S)
        ent = Runner(nc)
        _PROG_CACHE[key] = ent
    return ent


# --------------------------------------------------------------------------
# kernel
# --------------------------------------------------------------------------

def kernel(**inputs):
    global LAST_EXEC_NS
    t_start = time.time()

    x = np.asarray(inputs["x"], np.float32)
    ei = np.asarray(inputs["edge_index"])
    ew = np.asarray(inputs["edge_weight"], np.float32)
    W1 = np.asarray(inputs["W1"], np.float32)
    We1 = np.asarray(inputs["We1"], np.float32)
    as1 = np.asarray(inputs["as1"], np.float32)
    ad1 = np.asarray(inputs["ad1"], np.float32)
    ae1 = np.asarray(inputs["ae1"], np.float32)
    b1 = np.asarray(inputs["b1"], np.float32)
    W2 = np.asarray(inputs["W2"], np.float32)
    We2 = np.asarray(inputs["We2"], np.float32)
    as2 = np.asarray(inputs["as2"], np.float32)
    ad2 = np.asarray(inputs["ad2"], np.float32)
    ae2 = np.asarray(inputs["ae2"], np.float32)
    b2 = np.asarray(inputs["b2"], np.float32)

    n = x.shape[0]
    S = get_structure(ei, ew, n)
    runner = get_runner(S)
    NS, NSP, NC = S["NS"], S["NSP"], S["NC"]
    src, dst, ea = S["src"], S["dst"], S["ea"]

    dev = {}

    # static graph arrays — upload first (async)
    dev["gsrc"] = runner.put(S["gsrc_c"])
    dev["gdst"] = runner.put(S["gdst_c"])
    dev["dstl"] = runner.put(S["dstloc_c"])

    # x -> transposed bf16 shards (convert to bf16 first; transpose 2-byte)
    xb = x.astype(BF16_NP)
    xT = _hbuf("xT", (NCORES, 128, NSP), BF16_NP)
    xT[:, :, :NS] = xb.reshape(NCORES, NS, 128).transpose(0, 2, 1)
    if NSP > NS:
        xT[:, :, NS:] = 0
    dev["xT"] = runner.put(xT.reshape(NCORES * P, NSP))

    # per-edge attr (self-loop attrs included), fp16; empty slots stay 0 and
    # are masked on device by the dstloc=255 sentinel
    wal = S["wal_row"]
    eww = _hbuf("eww", (NCORES * P * NC * SLOTS,), np.float16)
    eww.fill(0)
    eww[wal] = ea[:, 0]
    dev["ew"] = runner.put(eww.reshape(NCORES * P, NC * SLOTS))

    # consts
    Ws1 = np.stack([W1[:, h * HID:(h + 1) * HID] @ as1[0, h]
                    for h in range(H1)], axis=1)           # [128, H1]
    Wd1 = np.stack([W1[:, h * HID:(h + 1) * HID] @ ad1[0, h]
                    for h in range(H1)], axis=1)
    ce1 = (We1.reshape(H1, HID) * ae1[0]).sum(-1)          # [H1]
    ce2 = float((We2.reshape(H2, OUT_DIM)[0] * ae2[0, 0]).sum())
    W1e = np.concatenate([W1, Ws1, Wd1], axis=1)           # [128, 128+2*H1]
    W1eb = np.tile(W1e.astype(BF16_NP), (NCORES, 1))
    Ws2 = W2 @ as2[0, 0]
    Wd2 = W2 @ ad2[0, 0]
    W2e = np.concatenate([W2, Ws2[:, None], Wd2[:, None]], axis=1)
    W2eb = np.tile(W2e.astype(BF16_NP), (NCORES, 1))
    b1rep = np.tile(b1[None, :], (NCORES * P, 1)).astype(np.float32)
    b2rep = np.tile(b2[None, :], (NCORES * P, 1)).astype(np.float32)
    iota = np.tile(np.arange(P, dtype=np.float32)[None, :],
                   (NCORES * P, 1)).astype(BF16_NP)
    identity = np.tile(np.eye(P, dtype=np.float32), (NCORES, 1)).astype(BF16_NP)
    csc = np.tile(np.array([ce1[0], ce1[1] if H1 > 1 else 0.0, ce2, 0.0],
                           np.float32)[None, :], (NCORES * P, 1))
    dev["W1e"] = runner.put(W1eb)
    dev["W2e"] = runner.put(W2eb)
    dev["b1r"] = runner.put(b1rep)
    dev["b2r"] = runner.put(b2rep)
    dev["iotaT"] = runner.put(iota)
    dev["ident"] = runner.put(identity)
    dev["cscal"] = runner.put(csc)

    outs = runner.run(dev)
    o = np.asarray(outs["out"])                            # [8*NSP, 64] f16
    res = o.reshape(NCORES, NSP, OUT_DIM)[:, :NS].reshape(n, OUT_DIM)
    res = res.astype(np.float32)

    LAST_EXEC_NS = int((time.time() - t_start) * 1e9)
    return res


# revision 18
# speedup vs baseline: 1.1005x; 1.1005x over previous
"""GAT (2-layer) Trainium2 Bass kernel, 8-core SPMD — fused single-launch.

v3 design (axon tunnel is ~80MB/s up / ~50MB/s down, so bytes-over-tunnel and
per-call recompilation dominate; device exec is ~ms):
- ONE fused device program runs both GAT layers: L1 projection (node-sharded)
  -> AllGather h1 table -> L1 edge pass (dst-sharded scatter via one-hot
  matmuls) -> ELU -> L2 projection (+ attention src/dst terms as extra table
  columns) -> AllGather h2 table -> L2 edge pass -> fp16 output shard.
- ALL attention logits are computed on device. L1: asn1/adn1 are extra
  projection columns stored in a second gather table (asntab1, 256B rows);
  asn1[src] comes via the src gather, adn1[dst] via a local-table gather
  keyed by dst. L2: asn2/adn2 ride in the h2 table rows (cols 64/65). The
  raw per-edge attr ew (shared by both layers) is the only per-edge float
  upload (fp16); per-head We*ae coefficients arrive via a tiny cscal tensor.
- Empty padding slots carry dstloc=255: their one-hot scatter column matches
  nothing, so phantom edges contribute exactly zero regardless of logits.
- Edge scheduling: edges sharded by dst node-range; per core ordered by
  (src 32k-window group, dst 128-block); padded to a compile-time-identical
  tile/chunk structure across cores. Structure + compiled program + jitted
  executable + host staging buffers are cached across calls keyed on a hash
  of (edge_index, edge_weight).
- Per-call tunnel traffic: ~45MB up, ~13MB down (vs ~530MB/76MB for the
  two-launch replicated design). Warm wall-clock ~1.0s vs 25.3s baseline.
"""

import time
import hashlib
from concurrent.futures import ThreadPoolExecutor
import numpy as np
from contextlib import ExitStack

import jax
import jax.numpy as jnp
from jax.sharding import Mesh, PartitionSpec, NamedSharding

from jax.experimental.shard_map import shard_map

import concourse.bass as bass
import concourse.tile as tile
from concourse import bacc, mybir, bass2jax

F32 = mybir.dt.float32
BF16 = mybir.dt.bfloat16
F16 = mybir.dt.float16
I16 = mybir.dt.int16
AF = mybir.ActivationFunctionType
ALU = mybir.AluOpType

BF16_NP = np.dtype("bfloat16")

NCORES = 8
P = 128
CHUNK = 8192
SLOTS = CHUNK // P          # 64
IDXF = CHUNK // 16          # 512
SRC_CHUNK = 32768
NEG = -60000.0              # "minus infinity" that fits fp16

# problem constants
HID = 64
OUT_DIM = 64
H1, H2 = 2, 1

LAST_EXEC_NS = None


def _ceil_to(x, m):
    return (x + m - 1) // m * m


# --------------------------------------------------------------------------
# host: graph structure (cached per edge_index)
# --------------------------------------------------------------------------

def build_structure(src, dst, n):
    """Vectorized edge scheduling. Returns dict with compile-time structure
    and ready-to-upload concat arrays for everything except per-layer edge
    values (al1/ewt), for which a scatter index is provided."""
    NS = n // NCORES
    assert NS * NCORES == n
    NSP = _ceil_to(NS, P)
    NB = NSP // P
    TBL = NCORES * NSP
    NG = (TBL + SRC_CHUNK - 1) // SRC_CHUNK
    E = src.shape[0]

    tid = (src // NS) * NSP + (src % NS)          # gather-table row of src
    grp = tid // SRC_CHUNK
    core = dst // NS
    dloc = dst % NS
    blk = dloc // P

    key = (core * NG + grp) * NB + blk
    counts = np.bincount(key, minlength=NCORES * NG * NB)
    cnt3 = counts.reshape(NCORES, NG, NB)
    tiles_gb = (-(-cnt3 // P)).max(axis=0)        # [NG, NB]
    ts = np.cumsum(tiles_gb, axis=1) - tiles_gb   # tile start within group
    gtiles = tiles_gb.sum(axis=1)
    gchunks = -(-gtiles // SLOTS)
    first_chunk = np.concatenate([[0], np.cumsum(gchunks)[:-1]])
    NC = int(gchunks.sum())

    order = np.argsort(key, kind="stable")
    ro = np.empty(E, np.int64)
    ro[order] = np.arange(E)
    bucket_start = np.concatenate([[0], np.cumsum(counts)])
    j = ro - bucket_start[key]                    # rank within (c,g,b) bucket
    slot = first_chunk[grp] * CHUNK + (ts[grp, blk] + j // P) * P + (j % P)

    FL = NC * CHUNK
    gsrc_flat = np.zeros((NCORES, FL), np.int16)
    gsrc_flat[core, slot] = (tid - grp * SRC_CHUNK).astype(np.int16)
    gdst_flat = np.zeros((NCORES, FL), np.int16)
    gdst_flat[core, slot] = dloc.astype(np.int16)
    # 255 = empty-slot sentinel: the one-hot scatter column matches nothing,
    # so phantom (padding) edges contribute zero regardless of their logits
    dstloc_flat = np.full((NCORES, FL), 255, np.uint8)
    dstloc_flat[core, slot] = (dloc % P).astype(np.uint8)

    def wrap_scalar(flat, width, dtype):
        a = flat.reshape(NCORES, NC, SLOTS, P, width)
        return np.ascontiguousarray(
            a.transpose(0, 3, 1, 2, 4).reshape(NCORES * P, NC * SLOTS * width)
        ).astype(dtype)

    def wrap_idx(flat):
        a = flat.reshape(NCORES, NC, IDXF, 16)
        return np.ascontiguousarray(
            a.transpose(0, 3, 1, 2).reshape(NCORES * 16, NC * IDXF))

    gsrc_c = wrap_idx(gsrc_flat)
    gdst_c = wrap_idx(gdst_flat)
    dstloc_c = wrap_scalar(dstloc_flat, 1, np.uint8)

    # per-edge row into the wrapped [NCORES*P*NC*SLOTS, width] layout
    k = slot // CHUNK
    r = slot % CHUNK
    s = r // P
    p = r % P
    wal_row = (core * P + p) * (NC * SLOTS) + k * SLOTS + s

    # compile-time tile descriptors
    chunk_group, chunk_tiles = [], []
    for q in range(NG):
        gts = []
        for b in range(NB):
            t = int(tiles_gb[q, b])
            for i in range(t):
                gts.append((b, i == 0, i == t - 1))
        gts += [None] * ((-len(gts)) % SLOTS)
        for i0 in range(0, len(gts), SLOTS):
            chunk_group.append(q)
            chunk_tiles.append(gts[i0:i0 + SLOTS])
    assert len(chunk_tiles) == NC

    sig = hashlib.blake2b(
        (repr((NS, NSP, NB, TBL, NG, NC, chunk_group, chunk_tiles))).encode(),
        digest_size=16).hexdigest()

    return dict(NS=NS, NSP=NSP, NB=NB, TBL=TBL, NG=NG, NC=NC,
                chunk_group=chunk_group, chunk_tiles=chunk_tiles,
                gsrc_c=gsrc_c, gdst_c=gdst_c, dstloc_c=dstloc_c,
                wal_row=wal_row, sig=sig)


_STRUCT_CACHE = {}


def get_structure(edge_index, ew, n):
    h = hashlib.blake2b(np.ascontiguousarray(edge_index).tobytes(),
                        digest_size=16)
    h.update(np.ascontiguousarray(ew).tobytes())
    hkey = h.digest() + n.to_bytes(8, "little")
    ent = _STRUCT_CACHE.get(hkey)
    if ent is None:
        src, dst, ea = add_self_loops_np(
            np.asarray(edge_index[0], np.int64),
            np.asarray(edge_index[1], np.int64), ew, n)
        S = build_structure(src, dst, n)
        S["src"], S["dst"], S["ea"] = src, dst, ea
        _STRUCT_CACHE[hkey] = S
        ent = S
    return ent


def add_self_loops_np(src, dst, ew, n):
    deg = np.bincount(dst, minlength=n).astype(np.float32)
    sw = np.bincount(dst, weights=ew[:, 0], minlength=n).astype(np.float32)
    loop = sw / np.maximum(deg, 1.0)
    ar = np.arange(n, dtype=src.dtype)
    return (np.concatenate([src, ar]), np.concatenate([dst, ar]),
            np.concatenate([ew, loop[:, None].astype(np.float32)], axis=0))


# --------------------------------------------------------------------------
# device program (fused 2 layers)
# --------------------------------------------------------------------------

def build_program(S):
    NSP, NB, TBL, NC = S["NSP"], S["NB"], S["TBL"], S["NC"]
    chunk_group, chunk_tiles = S["chunk_group"], S["chunk_tiles"]
    HC1 = H1 * HID            # 128
    HC2 = H2 * OUT_DIM        # 64
    RW1 = HC1 + H1            # 130
    RW2 = HC2 + H2            # 65

    nc = bacc.Bacc("TRN2", target_bir_lowering=False, debug=False,
                   num_devices=NCORES)

    xT = nc.dram_tensor("xT", [P, NSP], BF16, kind="ExternalInput")
    # W1e = [W1 | Ws1 (H1 cols) | Wd1 (H1 cols)] so the L1 projection also
    # produces per-node attention terms asn1/adn1
    W1e = nc.dram_tensor("W1e", [P, HC1 + 2 * H1], BF16, kind="ExternalInput")
    W2e = nc.dram_tensor("W2e", [P, HC2 + 2], BF16, kind="ExternalInput")
    b1r = nc.dram_tensor("b1r", [P, HC1], F32, kind="ExternalInput")
    b2r = nc.dram_tensor("b2r", [P, HC2], F32, kind="ExternalInput")
    iotaT = nc.dram_tensor("iotaT", [P, P], BF16, kind="ExternalInput")
    ident = nc.dram_tensor("ident", [P, P], BF16, kind="ExternalInput")
    cscal = nc.dram_tensor("cscal", [P, 4], F32, kind="ExternalInput")
    ew = nc.dram_tensor("ew", [P, NC * SLOTS], mybir.dt.uint8,
                    kind="ExternalInput")
    dstl = nc.dram_tensor("dstl", [P, NC * SLOTS], mybir.dt.uint8,
                      kind="ExternalInput")
    gsrc = nc.dram_tensor("gsrc", [16, NC * IDXF], I16, kind="ExternalInput")
    gdst = nc.dram_tensor("gdst", [16, NC * IDXF], I16, kind="ExternalInput")
    out = nc.dram_tensor("out", [NSP, HC2], F16, kind="ExternalOutput")

    htab1s = nc.dram_tensor("htab1s", [NSP, 128], BF16, kind="Internal")
    htab1 = nc.dram_tensor("htab1", [TBL, 128], BF16, kind="Internal",
                           addr_space="Shared")
    asntab1s = nc.dram_tensor("asntab1s", [NSP, 128], BF16, kind="Internal")
    asntab1 = nc.dram_tensor("asntab1", [TBL, 128], BF16, kind="Internal",
                             addr_space="Shared")
    htab2s = nc.dram_tensor("htab2s", [NSP, 128], BF16, kind="Internal")
    htab2 = nc.dram_tensor("htab2", [TBL, 128], BF16, kind="Internal",
                           addr_space="Shared")
    eluD = nc.dram_tensor("eluD", [NSP, HC1], BF16, kind="Internal")
    gsrcr = nc.dram_tensor("gsrcr", [P, NC * IDXF], I16, kind="Internal")
    gdstr = nc.dram_tensor("gdstr", [P, NC * IDXF], I16, kind="Internal")

    with ExitStack() as ctx:
        tc = ctx.enter_context(tile.TileContext(nc))
        cpool = ctx.enter_context(tc.tile_pool(name="const", bufs=1))
        W1_sb = cpool.tile([P, HC1 + 2 * H1], BF16)
        nc.sync.dma_start(W1_sb[:], W1e.ap())
        W2e_sb = cpool.tile([P, HC2 + 2], BF16)
        nc.sync.dma_start(W2e_sb[:], W2e.ap())
        b1_sb = cpool.tile([P, 1, HC1], F32)
        nc.sync.dma_start(b1_sb[:, 0, :], b1r.ap())
        b2_sb = cpool.tile([P, 1, HC2], F32)
        nc.sync.dma_start(b2_sb[:, 0, :], b2r.ap())
        iota_sb = cpool.tile([P, 1, P], BF16)
        nc.sync.dma_start(iota_sb[:, 0, :], iotaT.ap())
        id_sb = cpool.tile([P, P], BF16)
        nc.sync.dma_start(id_sb[:], ident.ap())
        csc_sb = cpool.tile([P, 1, 4], F32)
        nc.sync.dma_start(csc_sb[:, 0, :], cscal.ap())
        acc_sb = cpool.tile([P, NB, RW1], F32)

        # replicate compact idx arrays to 128 partitions (DRAM->DRAM)
        for rr in range(8):
            nc.sync.dma_start(gsrcr.ap()[rr * 16:(rr + 1) * 16, :], gsrc.ap())
            nc.sync.dma_start(gdstr.ap()[rr * 16:(rr + 1) * 16, :], gdst.ap())

        xpool = ctx.enter_context(tc.tile_pool(name="xp", bufs=3))
        hpool = ctx.enter_context(tc.tile_pool(name="hp", bufs=3))
        pspool = ctx.enter_context(tc.tile_pool(name="ps", bufs=4,
                                                space="PSUM"))
        ipool = ctx.enter_context(tc.tile_pool(name="ip", bufs=3))
        gpool = ctx.enter_context(tc.tile_pool(name="gp", bufs=3))
        apool = ctx.enter_context(tc.tile_pool(name="apl", bufs=3))
        epool = ctx.enter_context(tc.tile_pool(name="ep", bufs=3))
        spool = ctx.enter_context(tc.tile_pool(name="sp", bufs=2))
        rpool = ctx.enter_context(tc.tile_pool(name="rp", bufs=2))
        mpool = ctx.enter_context(tc.tile_pool(name="mp", bufs=4,
                                               space="PSUM"))
        fpool = ctx.enter_context(tc.tile_pool(name="fp", bufs=2))

        # ---------------- phase A: L1 projection of own shard --------------
        # h1ext = x @ [W1|Ws1|Wd1]: cols 0:HC1 -> htab1s, HC1:HC1+4
        # (asn1/adn1 per head) -> asntab1s
        BK = 4
        NA = 2 * H1
        for b0 in range(0, NB, BK):
            kk = min(BK, NB - b0)
            xt = xpool.tile([P, BK * P], BF16, tag="xt", bufs=2)
            nc.sync.dma_start(xt[:, 0:kk * P], xT.ap()[:, b0 * P:(b0 + kk) * P])
            hs = hpool.tile([P, BK, 128], BF16, tag="hs", bufs=2)
            asnt = hpool.tile([P, BK, 128], BF16, tag="asnt", bufs=2)
            nc.vector.memset(asnt[:, 0:kk, NA:128], 0.0)
            for i in range(kk):
                ps = pspool.tile([P, HC1 + NA], F32, tag="ps", bufs=4)
                nc.tensor.matmul(ps[:], xt[:, i * P:(i + 1) * P], W1_sb[:],
                                 start=True, stop=True)
                nc.scalar.activation(hs[:, i, :], ps[:, 0:HC1], AF.Copy)
                nc.scalar.activation(asnt[:, i, 0:NA], ps[:, HC1:HC1 + NA],
                                     AF.Copy)
            nc.sync.dma_start(
                htab1s.ap()[b0 * P:(b0 + kk) * P, :].rearrange(
                    "(k p) t -> p k t", p=P),
                hs[:, 0:kk, :])
            nc.sync.dma_start(
                asntab1s.ap()[b0 * P:(b0 + kk) * P, :].rearrange(
                    "(k p) t -> p k t", p=P),
                asnt[:, 0:kk, :])

        nc.gpsimd.collective_compute(
            "AllGather", ALU.bypass,
            replica_groups=[list(range(NCORES))],
            ins=[htab1s[:].opt()], outs=[htab1[:].opt()])
        nc.gpsimd.collective_compute(
            "AllGather", ALU.bypass,
            replica_groups=[list(range(NCORES))],
            ins=[asntab1s[:].opt()], outs=[asntab1[:].opt()])

        # ---------------- edge pass helper ---------------------------------
        def edge_phase(layer):
            hc = HC1 if layer == 1 else HC2
            nh = H1 if layer == 1 else H2
            hd = hc // nh
            rw = hc + nh
            tab = htab1 if layer == 1 else htab2
            cur = [None]

            def close_run():
                if cur[0] is not None:
                    pst, bb = cur[0]
                    nc.vector.tensor_add(acc_sb[:, bb, 0:rw],
                                         acc_sb[:, bb, 0:rw], pst[:, 0:rw])
                    cur[0] = None

            for ck in range(NC):
                q = chunk_group[ck]
                r0 = q * SRC_CHUNK
                r1 = min(r0 + SRC_CHUNK, TBL)
                gi = ipool.tile([P, IDXF], I16, tag="gi", bufs=3)
                nc.sync.dma_start(gi[:],
                                  gsrcr.ap()[:, ck * IDXF:(ck + 1) * IDXF])
                grows = gpool.tile([P, SLOTS, 128], BF16, tag="grows", bufs=3)
                nc.gpsimd.dma_gather(grows[:], tab.ap()[r0:r1, :], gi[:],
                                     num_idxs=CHUNK, num_idxs_reg=CHUNK,
                                     elem_size=128, single_packet=False)
                e8 = apool.tile([P, SLOTS, 1], mybir.dt.uint8)
                nc.sync.dma_start(
                    e8[:, :, 0], ew.ap()[:, ck * SLOTS:(ck + 1) * SLOTS])
                ewf = epool.tile([P, SLOTS, 1], F32)
                nc.vector.tensor_copy(ewf[:], e8[:])
                if layer == 1:
                    # asn1[src] via src-gather of asntab1; adn1[dst] via
                    # local-table gather keyed by dst
                    ga_s = gpool.tile([P, SLOTS, 128], BF16, tag="grows",
                                      bufs=3)
                    nc.gpsimd.dma_gather(ga_s[:], asntab1.ap()[r0:r1, :],
                                         gi[:], num_idxs=CHUNK,
                                         num_idxs_reg=CHUNK, elem_size=128,
                                         single_packet=False)
                    gid = ipool.tile([P, IDXF], I16, tag="gi", bufs=3)
                    nc.sync.dma_start(
                        gid[:], gdstr.ap()[:, ck * IDXF:(ck + 1) * IDXF])
                    ga_d = gpool.tile([P, SLOTS, 128], BF16, tag="grows",
                                      bufs=3)
                    nc.gpsimd.dma_gather(ga_d[:], asntab1s.ap()[0:NSP, :],
                                         gid[:], num_idxs=CHUNK,
                                         num_idxs_reg=CHUNK, elem_size=128,
                                         single_packet=False)
                    alf = epool.tile([P, SLOTS, nh], F32)
                    nc.vector.tensor_copy(alf[:], ga_s[:, :, 0:nh])
                    t_adn = epool.tile([P, SLOTS, nh], F32)
                    nc.vector.tensor_copy(t_adn[:], ga_d[:, :, nh:2 * nh])
                    nc.vector.tensor_add(alf[:], alf[:], t_adn[:])
                    ewc = epool.tile([P, SLOTS, nh], F32)
                    e1a, e2a = bass.broadcast_tensor_aps(
                        ewf[:], csc_sb[:, :, 0:nh])
                    nc.vector.tensor_mul(ewc[:], e1a, e2a)
                    nc.vector.tensor_add(alf[:], alf[:], ewc[:])
                else:
                    gid = ipool.tile([P, IDXF], I16, tag="gi", bufs=3)
                    nc.sync.dma_start(
                        gid[:], gdstr.ap()[:, ck * IDXF:(ck + 1) * IDXF])
                    growsd = gpool.tile([P, SLOTS, 128], BF16, tag="grows", bufs=3)
                    nc.gpsimd.dma_gather(growsd[:], htab2s.ap()[0:NSP, :],
                                         gid[:], num_idxs=CHUNK,
                                         num_idxs_reg=CHUNK, elem_size=128,
                                         single_packet=False)
                    alf = epool.tile([P, SLOTS, 1], F32)
                    e1a, e2a = bass.broadcast_tensor_aps(
                        ewf[:], csc_sb[:, :, 2:3])
                    nc.vector.tensor_mul(alf[:], e1a, e2a)
                    t_asn = epool.tile([P, SLOTS, 1], F32)
                    nc.vector.tensor_copy(t_asn[:], grows[:, :, hc:hc + 1])
                    nc.vector.tensor_add(alf[:], alf[:], t_asn[:])
                    t_adn = epool.tile([P, SLOTS, 1], F32)
                    nc.vector.tensor_copy(t_adn[:], growsd[:, :, hc + 1:hc + 2])
                    nc.vector.tensor_add(alf[:], alf[:], t_adn[:])

                # ex = exp(leaky_relu(al))
                t1 = epool.tile([P, SLOTS, nh], F32)
                nc.vector.tensor_scalar_mul(t1[:], alf[:], 0.2)
                nc.vector.tensor_max(t1[:], t1[:], alf[:])
                ex = epool.tile([P, SLOTS, nh], BF16)
                nc.scalar.activation(ex[:], t1[:], AF.Exp)

                dl8 = apool.tile([P, SLOTS, 1], mybir.dt.uint8)
                nc.sync.dma_start(dl8[:, :, 0],
                                  dstl.ap()[:, ck * SLOTS:(ck + 1) * SLOTS])
                dlt = apool.tile([P, SLOTS, 1], BF16)
                nc.vector.tensor_copy(dlt[:], dl8[:])
                sw = spool.tile([P, SLOTS, P], BF16)
                a1, a2 = bass.broadcast_tensor_aps(iota_sb[:], dlt[:])
                nc.vector.tensor_tensor(sw[:], a1, a2, ALU.is_equal)

                rhs = rpool.tile([P, SLOTS, rw], BF16)
                for h in range(nh):
                    b1a, b2a = bass.broadcast_tensor_aps(
                        grows[:, :, h * hd:(h + 1) * hd], ex[:, :, h:h + 1])
                    nc.vector.tensor_mul(rhs[:, :, h * hd:(h + 1) * hd],
                                         b1a, b2a)
                nc.vector.tensor_copy(rhs[:, :, hc:hc + nh], ex[:])

                for s in range(SLOTS):
                    td = chunk_tiles[ck][s]
                    if td is None:
                        continue
                    bb, st, sp = td
                    if st:
                        close_run()
                        pst = mpool.tile([P, RW1], F32, tag="pst", bufs=4)
                        cur[0] = (pst, bb)
                    else:
                        pst, _ = cur[0]
                    nc.tensor.matmul(pst[:, 0:rw], sw[:, s, :], rhs[:, s, :],
                                     start=st, stop=sp)
            close_run()

        # ---------------- phase B: L1 edges --------------------------------
        nc.vector.memset(acc_sb[:], 0.0)
        edge_phase(1)

        # finalize L1: out1 = num/den + b1, ELU, -> eluD (bf16)
        FB = 4
        for b0 in range(0, NB, FB):
            kf = min(FB, NB - b0)
            rec = fpool.tile([P, FB, H1], F32)
            nc.vector.tensor_scalar_add(
                rec[:, 0:kf, :], acc_sb[:, b0:b0 + kf, HC1:HC1 + H1], 1e-30)
            nc.vector.reciprocal(rec[:, 0:kf, :], rec[:, 0:kf, :])
            outt = fpool.tile([P, FB, HC1], F32)
            for h in range(H1):
                c1, c2 = bass.broadcast_tensor_aps(
                    acc_sb[:, b0:b0 + kf, h * HID:(h + 1) * HID],
                    rec[:, 0:kf, h:h + 1])
                nc.vector.tensor_mul(outt[:, 0:kf, h * HID:(h + 1) * HID],
                                     c1, c2)
            d1, d2 = bass.broadcast_tensor_aps(outt[:, 0:kf, :], b1_sb[:])
            nc.vector.tensor_add(outt[:, 0:kf, :], d1, d2)
            # ELU
            neg = fpool.tile([P, FB, HC1], F32)
            nc.vector.tensor_scalar_min(neg[:, 0:kf, :], outt[:, 0:kf, :], 0.0)
            enx = fpool.tile([P, FB, HC1], F32)
            nc.scalar.activation(enx[:, 0:kf, :], neg[:, 0:kf, :], AF.Exp)
            nc.vector.tensor_scalar_add(enx[:, 0:kf, :], enx[:, 0:kf, :], -1.0)
            nc.vector.tensor_scalar_max(outt[:, 0:kf, :], outt[:, 0:kf, :],
                                        0.0)
            nc.vector.tensor_add(outt[:, 0:kf, :], outt[:, 0:kf, :],
                                 enx[:, 0:kf, :])
            e16 = fpool.tile([P, FB, HC1], BF16)
            nc.vector.tensor_copy(e16[:, 0:kf, :], outt[:, 0:kf, :])
            nc.sync.dma_start(
                eluD.ap()[b0 * P:(b0 + kf) * P, :].rearrange(
                    "(k p) c -> p k c", p=P),
                e16[:, 0:kf, :])

        # ---------------- phase C: L2 projection ---------------------------
        for b in range(NB):
            et = xpool.tile([P, P], BF16)
            nc.sync.dma_start(
                et[:],
                eluD.ap()[b * P:(b + 1) * P, :].rearrange(
                    "(k p) c -> p (k c)", p=P))
            psT = pspool.tile([P, 128], F32, tag="ps", bufs=4)
            nc.tensor.matmul(psT[:], et[:], id_sb[:], start=True, stop=True)
            etT = hpool.tile([P, P], BF16)
            nc.scalar.activation(etT[:], psT[:], AF.Copy)
            ps2 = pspool.tile([P, 128], F32, tag="ps", bufs=4)
            nc.tensor.matmul(ps2[:, 0:HC2 + 2], etT[:], W2e_sb[:],
                             start=True, stop=True)
            hs2 = hpool.tile([P, 128], BF16)
            nc.vector.memset(hs2[:, HC2 + 2:128], 0.0)
            nc.scalar.activation(hs2[:, 0:HC2 + 2], ps2[:, 0:HC2 + 2],
                                 AF.Copy)
            nc.sync.dma_start(
                htab2s.ap()[b * P:(b + 1) * P, :].rearrange(
                    "(k p) t -> p (k t)", p=P),
                hs2[:])

        nc.gpsimd.collective_compute(
            "AllGather", ALU.bypass,
            replica_groups=[list(range(NCORES))],
            ins=[htab2s[:].opt()], outs=[htab2[:].opt()])

        # ---------------- phase D: L2 edges --------------------------------
        nc.vector.memset(acc_sb[:, :, 0:RW2], 0.0)
        edge_phase(2)

        # finalize L2: out = num/den + b2 -> fp16
        for b0 in range(0, NB, FB):
            kf = min(FB, NB - b0)
            rec = fpool.tile([P, FB, H2], F32)
            nc.vector.tensor_scalar_add(
                rec[:, 0:kf, :], acc_sb[:, b0:b0 + kf, HC2:HC2 + H2], 1e-30)
            nc.vector.reciprocal(rec[:, 0:kf, :], rec[:, 0:kf, :])
            outt = fpool.tile([P, FB, HC2], F32)
            c1, c2 = bass.broadcast_tensor_aps(
                acc_sb[:, b0:b0 + kf, 0:HC2], rec[:, 0:kf, 0:1])
            nc.vector.tensor_mul(outt[:, 0:kf, :], c1, c2)
            d1, d2 = bass.broadcast_tensor_aps(outt[:, 0:kf, :], b2_sb[:])
            nc.vector.tensor_add(outt[:, 0:kf, :], d1, d2)
            o16 = fpool.tile([P, FB, HC2], F16)
            nc.vector.tensor_copy(o16[:, 0:kf, :], outt[:, 0:kf, :])
            nc.sync.dma_start(
                out.ap()[b0 * P:(b0 + kf) * P, :].rearrange(
                    "(k p) c -> p k c", p=P),
                o16[:, 0:kf, :])

    nc.compile()
    return nc


# --------------------------------------------------------------------------
# persistent-jit runner
# --------------------------------------------------------------------------

class Runner:
    def __init__(self, nc):
        bass2jax.install_neuronx_cc_hook()
        self.nc = nc
        partition_name = (nc.partition_id_tensor.name
                          if nc.partition_id_tensor else None)
        in_names, out_names, out_avals = [], [], []
        for alloc in nc.m.functions[0].allocations:
            if not isinstance(alloc, mybir.MemoryLocationSet):
                continue
            name = alloc.memorylocations[0].name
            if alloc.kind == "ExternalInput":
                if name != partition_name:
                    in_names.append(name)
            elif alloc.kind == "ExternalOutput":
                out_names.append(name)
                out_avals.append(jax.core.ShapedArray(
                    tuple(alloc.tensor_shape), mybir.dt.np(alloc.dtype)))
        self.in_names = in_names
        self.out_names = out_names
        self.out_avals = out_avals
        n_params = len(in_names)
        n_outs = len(out_avals)
        all_in = list(in_names) + list(out_names)
        if partition_name is not None:
            all_in.append(partition_name)
        donate = tuple(range(n_params, n_params + n_outs))

        def _body(*args):
            operands = list(args)
            if partition_name is not None:
                operands.append(bass2jax.partition_id_tensor())
            outs = bass2jax._bass_exec_p.bind(
                *operands,
                out_avals=tuple(out_avals),
                in_names=tuple(all_in),
                out_names=tuple(out_names),
                lowering_input_output_aliases=(),
                sim_require_finite=True,
                sim_require_nnan=True,
                nc=nc,
            )
            return tuple(outs)

        devices = jax.devices()[:NCORES]
        self.mesh = Mesh(np.asarray(devices), ("core",))
        self.sharding = NamedSharding(self.mesh, PartitionSpec("core"))
        in_specs = (PartitionSpec("core"),) * (n_params + n_outs)
        out_specs = (PartitionSpec("core"),) * n_outs
        self.fn = jax.jit(
            shard_map(_body, mesh=self.mesh, in_specs=in_specs,
                      out_specs=out_specs, check_rep=False),
            donate_argnums=donate, keep_unused=True)
        self._zeros_fns = [
            jax.jit(lambda av=av: jnp.zeros((NCORES * av.shape[0],)
                                            + av.shape[1:], av.dtype),
                    out_shardings=self.sharding)
            for av in out_avals]

    def put(self, arr):
        """Async host->device upload with core sharding on axis 0."""
        return jax.device_put(arr, self.sharding)

    def run(self, concat_map):
        args = [concat_map[nm] for nm in self.in_names]
        zeros = getattr(self, "_zeros_next", None)
        if zeros is None:
            zeros = [zf() for zf in self._zeros_fns]
        outs = self.fn(*args, *zeros)
        # stage fresh donated zero buffers for the next call (async)
        self._zeros_next = [zf() for zf in self._zeros_fns]
        return {nm: outs[i] for i, nm in enumerate(self.out_names)}


_PROG_CACHE = {}
_HOST_BUFS = {}


def _hbuf(key, shape, dtype):
    """Reusable host staging buffer (avoids per-call alloc + page faults)."""
    b = _HOST_BUFS.get(key)
    if b is None or b.shape != tuple(shape) or b.dtype != dtype:
        b = np.empty(shape, dtype)
        _HOST_BUFS[key] = b
    return b


def get_runner(S):
    key = S["sig"]
    ent = _PROG_CACHE.get(key)
    if ent is None:
        nc = build_program(S)
        ent = Runner(nc)
        _PROG_CACHE[key] = ent
    return ent


# --------------------------------------------------------------------------
# kernel
# --------------------------------------------------------------------------

def kernel(**inputs):
    global LAST_EXEC_NS
    t_start = time.time()

    x = np.asarray(inputs["x"], np.float32)
    ei = np.asarray(inputs["edge_index"])
    ew = np.asarray(inputs["edge_weight"], np.float32)
    W1 = np.asarray(inputs["W1"], np.float32)
    We1 = np.asarray(inputs["We1"], np.float32)
    as1 = np.asarray(inputs["as1"], np.float32)
    ad1 = np.asarray(inputs["ad1"], np.float32)
    ae1 = np.asarray(inputs["ae1"], np.float32)
    b1 = np.asarray(inputs["b1"], np.float32)
    W2 = np.asarray(inputs["W2"], np.float32)
    We2 = np.asarray(inputs["We2"], np.float32)
    as2 = np.asarray(inputs["as2"], np.float32)
    ad2 = np.asarray(inputs["ad2"], np.float32)
    ae2 = np.asarray(inputs["ae2"], np.float32)
    b2 = np.asarray(inputs["b2"], np.float32)

    n = x.shape[0]
    S = get_structure(ei, ew, n)
    runner = get_runner(S)
    NS, NSP, NC = S["NS"], S["NSP"], S["NC"]
    src, dst, ea = S["src"], S["dst"], S["ea"]

    dev = {}

    # static graph arrays — upload first (async)
    dev["gsrc"] = runner.put(S["gsrc_c"])
    dev["gdst"] = runner.put(S["gdst_c"])
    dev["dstl"] = runner.put(S["dstloc_c"])

    # x -> transposed bf16 shards (convert to bf16 first; transpose 2-byte;
    # per-shard in threads — the copies release the GIL)
    xb = x.astype(BF16_NP).reshape(NCORES, NS, 128)
    xT = _hbuf("xT", (NCORES, 128, NSP), BF16_NP)

    def _xt_shard(c):
        xT[c, :, :NS] = xb[c].T
        if NSP > NS:
            xT[c, :, NS:] = 0
    with ThreadPoolExecutor(4) as _pool:
        list(_pool.map(_xt_shard, range(NCORES)))
    dev["xT"] = runner.put(xT.reshape(NCORES * P, NSP))

    # per-edge attr (self-loop attrs included), fp16; empty slots stay 0 and
    # are masked on device by the dstloc=255 sentinel
    wal = S["wal_row"]
    eww = _hbuf("eww", (NCORES * P * NC * SLOTS,), np.uint8)
    eww.fill(0)
    eww[wal] = np.rint(ea[:, 0] * 255.0).astype(np.uint8)
    dev["ew"] = runner.put(eww.reshape(NCORES * P, NC * SLOTS))

    # consts
    Ws1 = np.stack([W1[:, h * HID:(h + 1) * HID] @ as1[0, h]
                    for h in range(H1)], axis=1)           # [128, H1]
    Wd1 = np.stack([W1[:, h * HID:(h + 1) * HID] @ ad1[0, h]
                    for h in range(H1)], axis=1)
    ce1 = (We1.reshape(H1, HID) * ae1[0]).sum(-1)          # [H1]
    ce2 = float((We2.reshape(H2, OUT_DIM)[0] * ae2[0, 0]).sum())
    W1e = np.concatenate([W1, Ws1, Wd1], axis=1)           # [128, 128+2*H1]
    W1eb = np.tile(W1e.astype(BF16_NP), (NCORES, 1))
    Ws2 = W2 @ as2[0, 0]
    Wd2 = W2 @ ad2[0, 0]
    W2e = np.concatenate([W2, Ws2[:, None], Wd2[:, None]], axis=1)
    W2eb = np.tile(W2e.astype(BF16_NP), (NCORES, 1))
    b1rep = np.tile(b1[None, :], (NCORES * P, 1)).astype(np.float32)
    b2rep = np.tile(b2[None, :], (NCORES * P, 1)).astype(np.float32)
    iota = np.tile(np.arange(P, dtype=np.float32)[None, :],
                   (NCORES * P, 1)).astype(BF16_NP)
    identity = np.tile(np.eye(P, dtype=np.float32), (NCORES, 1)).astype(BF16_NP)
    csc = np.tile(np.array([ce1[0], ce1[1] if H1 > 1 else 0.0, ce2, 0.0],
                           np.float32)[None, :] / 255.0, (NCORES * P, 1))
    dev["W1e"] = runner.put(W1eb)
    dev["W2e"] = runner.put(W2eb)
    dev["b1r"] = runner.put(b1rep)
    dev["b2r"] = runner.put(b2rep)
    dev["iotaT"] = runner.put(iota)
    dev["ident"] = runner.put(identity)
    dev["cscal"] = runner.put(csc)

    outs = runner.run(dev)
    o = np.asarray(outs["out"])                            # [8*NSP, 64] f16
    res = o.reshape(NCORES, NSP, OUT_DIM)[:, :NS].reshape(n, OUT_DIM)
    res = res.astype(np.float32)

    LAST_EXEC_NS = int((time.time() - t_start) * 1e9)
    return res
